# revision 57
# baseline (speedup 1.0000x reference)
"""Trainium2 Bass kernel for a circular-padded 3x3 conv cellular-automaton step.

Computation (per image):
    z   = conv3x3_circular(x, Wc) ;  Wc = w1 @ w_perc  (host-fused, [96,12,3,3])
    h   = relu(z + b1)
    u   = w2 @ h + b2
    out = x + (mask > 0.5) * u

Mapping (per core, B=16 split 8 ways -> 2 images/core):
  * conv as ONE K=108 matmul per image row: partitions (dj, di, c);
    di row-shifts loaded 3x from DRAM, dj column shifts (1, 2) produced by
    two on-chip offset copies into partitions 36:72 and 72:108.
  * software-pipelined supertile loop: PE does [4 conv MMs of supertile t]
    then [4 col-tiled w2p MMs of supertile t-2], so relu latency is hidden
    and the PE streams warm (~163ns/MM) with LDWEIGHTS overlapped.
  * relu+bias split ScalarE (2 rows) / VectorE (1 row) / GPSIMD (1 row);
    ht has a constant ones-row (row 96) so b2 rides in the matmul.
  * mask multiply on VectorE against a prelaid [128, 96*384] bf16 mask,
    +x on GPSIMD, bf16 output staged per chunk, one padded DMA per chunk.
"""

import sys

if "/opt/trn_rl_repo" not in sys.path:
    sys.path.insert(0, "/opt/trn_rl_repo")

from contextlib import ExitStack

import numpy as np
import ml_dtypes

import concourse.bass as bass
import concourse.tile as tile
from concourse import mybir
from concourse.bass_utils import run_bass_kernel_spmd

B, C, H, W = 16, 12, 384, 384
CH = 96                      # hidden channels
NCORES = 8
BLOC = B // NCORES           # images per core
W2 = W + 2                   # circular-padded row length
PADH = H + 4                 # padded rows: 1 top + 3 bottom
CHUNK = 16                   # image rows per processing chunk
ST = 4                       # rows per supertile (one per PE column group)
NCHUNK = H // CHUNK
NST = CHUNK // ST            # supertiles per chunk
XQLEN = CHUNK * W2           # used free length per chunk
XBLEN = (CHUNK + 2) * W2     # loaded free length (halo + shift spill)
XPLEN = (H + 2) * W2 - 2     # flat tap-shifted x plane length per partition
MTILES = H // ST             # 96 supertile row-groups per image
STW = NST * W                # supertile-layout free length per chunk
NT = BLOC * NCHUNK * NST     # total supertiles per core

_BF16 = mybir.dt.bfloat16
_F32 = mybir.dt.float32
_FP8 = mybir.dt.float8e4


def _spill_waits(nc):
    """walrus/trn2 here accepts at most ONE sync-wait per instruction; move
    excess waits onto NoOps inserted immediately before, on the same engine."""
    nspill = 0
    for bbwrap in list(nc.bb_map.values()):
        bb = bbwrap.bb
        out = []
        for inst in bb.instructions:
            si = inst.sync_info
            if si is not None and si.on_wait and len(si.on_wait) > 1:
                waits = list(si.on_wait)
                for w in waits[1:]:
                    nop = mybir.InstNoOp(
                        name=nc.get_next_instruction_name(),
                        engine=inst.engine,
                        sync_info=mybir.SyncInfo(on_wait=[w], on_update=[]),
                        bass_nofuse=True,
                    )
                    nc.register_instruction(nop)
                    out.append(nop)
                    nspill += 1
                si.on_wait = waits[:1]
            out.append(inst)
        try:
            bb.instructions = out
        except Exception:
            bb.instructions.clear()
            bb.instructions.extend(out)
    return nspill


def _build_nc(reps=1):
    nc = bass.Bass()

    xpad = nc.declare_dram_parameter("xpad", [BLOC, 108, XPLEN], _FP8, isOutput=False)
    xstc = nc.declare_dram_parameter("xstc", [BLOC, NCHUNK // 2, ST, C, 2 * STW], _BF16, isOutput=False)
    wa = nc.declare_dram_parameter("wa", [108, CH], _BF16, isOutput=False)
    w2p = nc.declare_dram_parameter("w2p", [CH + 1, 32], _BF16, isOutput=False)
    b1 = nc.declare_dram_parameter("b1", [CH, 1], _F32, isOutput=False)
    m128 = nc.declare_dram_parameter("m128", [ST, 32, MTILES * W], _FP8, isOutput=False)
    out = nc.declare_dram_parameter("out", [BLOC, NCHUNK // 2, ST, C, 2 * STW], _BF16, isOutput=True)

    with tile.TileContext(nc) as tc, ExitStack() as ctx:
        state = _setup(ctx, tc, wa, w2p, b1, m128)
        if reps == 1:
            _loop_body(tc, state, xpad, xstc, out)
        else:
            with tc.For_i(0, reps, 1):
                _loop_body(tc, state, xpad, xstc, out)
    _spill_waits(nc)
    return nc


def _setup(ctx, tc, wa, w2p, b1, m128):
    nc = tc.nc

    const = ctx.enter_context(tc.tile_pool(name="const", bufs=1))
    ump = ctx.enter_context(tc.tile_pool(name="um", bufs=2))
    zp = ctx.enter_context(tc.tile_pool(name="z", bufs=3, space="PSUM"))
    up = ctx.enter_context(tc.tile_pool(name="u", bufs=2, space="PSUM"))

    wa_sb = const.tile([108, CH], _BF16)
    nc.sync.dma_start(out=wa_sb, in_=wa[:, :])
    w2p_sb = const.tile([CH + 1, 32], _BF16)
    nc.sync.dma_start(out=w2p_sb, in_=w2p[:, :])
    b1_sb = const.tile([CH, 1], _F32)
    nc.sync.dma_start(out=b1_sb, in_=b1[:, :])
    m128_sb = const.tile([128, MTILES * W], _FP8, name="m128_sb")
    MHALF = MTILES * W // 2
    for j in range(ST):
        for h in range(2):
            nc.sync.dma_start(
                out=m128_sb[32 * j : 32 * j + 32, h * MHALF : (h + 1) * MHALF],
                in_=m128[j, :, h * MHALF : (h + 1) * MHALF],
            )

    # manually multi-buffered tiles (stable addresses); only xt needs a
    # zero memset (its pad partitions are read by the gpsimd add)
    hts = [
        const.tile([CH + 1, ST, W], _BF16, name=f"ht{i}", tag=f"ht{i}")
        for i in range(4)
    ]
    xqs = [
        const.tile([108, XBLEN], _FP8, name=f"xqt{i}", tag=f"xqt{i}") for i in range(3)
    ]
    xts = [
        const.tile([128, 2 * STW], _BF16, name=f"xtt{i}", tag=f"xtt{i}")
        for i in range(3)
    ]
    ots = [
        const.tile([128, 2 * STW], _BF16, name=f"ott{i}", tag=f"ott{i}")
        for i in range(3)
    ]
    for t in xts:
        nc.vector.memset(t, 0.0)
    for ht in hts:
        nc.vector.memset(ht[CH : CH + 1, :, :], 1.0)

    # warmup matmuls: absorb weight-load DMA waits on the PE clock
    zw = zp.tile([CH, 2, 512], _F32, tag="z2")
    nc.tensor.matmul(zw[:, 0, 0:1], wa_sb, xqs[0][:, 0:1], start=True, stop=True)
    uw = up.tile([128, 512], _F32, tag="u")
    nc.tensor.matmul(
        uw[0:32, 0:1], w2p_sb, hts[0][:, 0, 0:1], start=True, stop=True,
        tile_position=(0, 0),
    )

    return dict(
        ump=ump, zp=zp, up=up,
        wa_sb=wa_sb, w2p_sb=w2p_sb, b1_sb=b1_sb, m128_sb=m128_sb,
        hts=hts, xqs=xqs, xts=xts, ots=ots,
    )


def _loop_body(tc, state, xpad, xstc, out):
    nc = tc.nc
    add = mybir.AluOpType.add
    mult = mybir.AluOpType.mult
    amax = mybir.AluOpType.max
    relu = mybir.ActivationFunctionType.Relu
    ump, zp, up = state["ump"], state["zp"], state["up"]
    wa_sb, w2p_sb, b1_sb, m128_sb = (
        state["wa_sb"], state["w2p_sb"], state["b1_sb"], state["m128_sb"],
    )
    hts, xqs, xts, ots = state["hts"], state["xqs"], state["xts"], state["ots"]

    NCHUNKS_TOT = BLOC * NCHUNK
    LAG = 3
    ELAG = 3

    def issue_xq_dmas(ci):
        b, chk = divmod(ci, NCHUNK)
        r0 = chk * CHUNK
        xq = xqs[ci % 3]
        # ONE plain-slice DMA for all 108 (dj, di, c) partitions: the host
        # pre-lays the nine tap-shifted flat copies, so this is a regular
        # [108, L] strided load that sprays across all 16 SDMA engines
        # (custom strided APs pin every line onto a single engine).
        nc.sync.dma_start(
            out=xq[0:108, 0 : XBLEN - 2],
            in_=xpad[b, :, r0 * W2 : r0 * W2 + XBLEN - 2],
        )

    NP = NCHUNK // 2

    def issue_xt_dmas(pair):
        # xt/ot tiles stage a PAIR of chunks (32 rows) to halve DMA
        # dispatch count on the SP queue. Issued at st==3 of the first
        # chunk of the previous pair: the lag-3 adds of pair-2 (last
        # reader of this buffer) are already issued then (issue-order
        # safety - Tile WAR tracking on these tiles is not reliable)
        b, pch = divmod(pair, NP)
        xt = xts[pair % 3]
        for j in range(ST):
            nc.sync.dma_start(
                out=xt[32 * j : 32 * j + C, :], in_=xstc[b, pch, j, :, :]
            )

    # z tiles rotate through the pool; remember per-supertile handles
    zpair = {}
    upair = {}

    issue_xq_dmas(0)
    issue_xq_dmas(1)
    issue_xt_dmas(0)

    for t in range(NT + ELAG):
        ci = t // NST
        st = t % NST

        if t < NT:
            xq = xqs[ci % 3]
            z2a = zp.tile([CH, 2, 512], _F32, tag="z2")
            z2b = zp.tile([CH, 2, 512], _F32, tag="z2")
            zpair[t] = (z2a, z2b)
            for j in range(ST):
                q = st * ST + j
                zt = (z2a if j < 2 else z2b)[:, j % 2, 0:W]
                nc.tensor.matmul(
                    zt, wa_sb, xq[0:108, q * W2 : q * W2 + W],
                    start=True, stop=True,
                )

        if LAG <= t < NT + LAG:
            tp = t - LAG
            ht = hts[tp % 4]
            u = up.tile([128, 512], _F32, tag="u")
            upair[tp] = u
            for j in range(ST):
                nc.tensor.matmul(
                    u[32 * j : 32 * j + 32, 0:W], w2p_sb, ht[:, j, :],
                    start=True, stop=True, tile_position=(0, 32 * j),
                )

        if t < NT:
            # relu for supertile t: ACT rows 0,1 + 3/4 of row 2; DVE the rest
            # (balances ACT ~1466ns vs DVE ~1465ns incl. the mask op)
            z2a, z2b = zpair.pop(t)
            ht = hts[t % 4]
            nc.scalar.activation(
                out=ht[0:CH, 0:2, :], in_=z2a[:, :, 0:W], func=relu, bias=b1_sb
            )
            nc.scalar.activation(
                out=ht[0:CH, 2:3, :], in_=z2b[:, 0:1, 0:W],
                func=relu, bias=b1_sb,
            )
            nc.vector.tensor_scalar(
                ht[0:CH, 3:4, :], z2b[:, 1:2, 0:W], b1_sb, 0.0, add, amax
            )

        # elementwise (mask + add + store) for supertile t-ELAG
        if ELAG <= t < NT + ELAG:
            tq = t - ELAG
            ciq = tq // NST
            stq = tq % NST
            bq, chkq = divmod(ciq, NCHUNK)
            pq = ciq // 2
            u = upair.pop(tq)
            xt = xts[pq % 3]
            ot = ots[pq % 3]
            seg = (ciq % 2) * STW + stq * W
            tglob = chkq * NST + stq
            um = ump.tile([128, W], _BF16)
            nc.vector.tensor_tensor(
                um, u[:, 0:W], m128_sb[:, tglob * W : tglob * W + W], mult
            )
            nc.gpsimd.tensor_tensor(
                ot[:, seg : seg + W], um, xt[:, seg : seg + W], add
            )
            if ciq % 2 == 1 and stq == NST - 1:
                for j in range(ST):
                    nc.sync.dma_start(
                        out=out[bq, chkq // 2, j, :, :],
                        in_=ot[32 * j : 32 * j + C, :],
                    )

        # prefetch DMAs last: every same-buffer reader from earlier
        # chunks is already issued at this point (issue-order safety —
        # Tile's WAR tracking on these manually-buffered tiles is not
        # reliable, see the st-pattern corruption bugs)
        if t < NT:
            # xq two chunks ahead at st==1: keeps this DMA AHEAD of the
            # out-write DMAs in SP-queue order (those wait on gpsimd adds
            # and head-of-line block the queue at pair boundaries)
            if st == 1 and ci + 2 < NCHUNKS_TOT:
                issue_xq_dmas(ci + 2)
            if st == 3 and ci % 2 == 0 and ci // 2 + 1 < BLOC * NP:
                issue_xt_dmas(ci // 2 + 1)


_NC_CACHE = {}


def _get_nc():
    if "nc" not in _NC_CACHE:
        _NC_CACHE["nc"] = _build_nc()
    return _NC_CACHE["nc"]


def _prep_inputs(x, w_perc, w1, b1, w2, b2, mask):
    bf16 = ml_dtypes.bfloat16
    wc = np.einsum("hp,pcij->hcij", w1, w_perc).astype(np.float32)  # [96,12,3,3]
    # wa[36*dj + 12*di + c, h] = wc[h, c, di, dj]
    wdjdic = wc.transpose(3, 2, 1, 0)  # [dj, di, c, h]
    wa = np.ascontiguousarray(wdjdic.reshape(108, CH)).astype(bf16)
    w2p = np.zeros((CH + 1, 32), np.float32)
    w2p[0:CH, 0:C] = w2.T
    w2p[CH, 0:C] = b2
    w2p = w2p.astype(bf16)
    b1c = np.ascontiguousarray(b1.reshape(CH, 1)).astype(np.float32)

    mbit = (mask > 0.5).astype(np.float32)
    # m128c[j, c, s*W+w] = mbit[4s+j, w] for c < C, zeros on pad partitions
    m128c = np.zeros((ST, 32, MTILES * W), ml_dtypes.float8_e4m3)
    m128c[:, 0:C, :] = np.broadcast_to(
        mbit.reshape(MTILES, ST, W).transpose(1, 0, 2).reshape(ST, 1, MTILES * W),
        (ST, C, MTILES * W),
    ).astype(ml_dtypes.float8_e4m3)

    xf8 = x.astype(ml_dtypes.float8_e4m3)
    in_maps = []
    for core in range(NCORES):
        xs = x[core * BLOC : (core + 1) * BLOC]
        xsp = np.pad(
            xf8[core * BLOC : (core + 1) * BLOC],
            ((0, 0), (0, 0), (1, 3), (1, 1)),
            mode="wrap",
        )
        # nine tap-shifted flat copies: xpad9[b, 36dj+12di+c, f] =
        # flat(xsp[b, c])[dj + di*W2 + f]
        xflat = xsp.reshape(BLOC, C, PADH * W2)
        xpad9 = np.empty((BLOC, 108, XPLEN), ml_dtypes.float8_e4m3)
        for dj in range(3):
            for di in range(3):
                off = dj + di * W2
                xpad9[:, 36 * dj + 12 * di : 36 * dj + 12 * di + C, :] = (
                    xflat[:, :, off : off + XPLEN]
                )
        # pair-staged supertile x:
        # xstc[b, p, j, c, h*STW + s*W + w] = x[b, c, 32p + 16h + 4s + j, w]
        NP = NCHUNK // 2
        xstc = np.ascontiguousarray(
            xs.reshape(BLOC, C, NP, 2, NST, ST, W).transpose(0, 2, 5, 1, 3, 4, 6)
        ).reshape(BLOC, NP, ST, C, 2 * STW).astype(bf16)
        in_maps.append(
            {
                "xpad": xpad9,
                "xstc": xstc,
                "wa": wa,
                "w2p": w2p,
                "b1": b1c,
                "m128": np.ascontiguousarray(m128c),
            }
        )
    return in_maps


def _unshard_out(core_outs):
    full = np.empty((B, C, H, W), np.float32)
    for core, o in enumerate(core_outs):
        NP = NCHUNK // 2
        o = np.asarray(o, np.float32).reshape(BLOC, NP, ST, C, 2, NST, W)
        # [b, p, j, c, h, s, w] -> [b, c, (p h s j), w]
        o = o.transpose(0, 3, 1, 4, 5, 2, 6).reshape(BLOC, C, H, W)
        full[core * BLOC : (core + 1) * BLOC] = o
    return full


def kernel(x, w_perc, w1, b1, w2, b2, mask):
    x = np.asarray(x, dtype=np.float32)
    in_maps = _prep_inputs(
        x,
        np.asarray(w_perc, np.float32),
        np.asarray(w1, np.float32),
        np.asarray(b1, np.float32),
        np.asarray(w2, np.float32),
        np.asarray(b2, np.float32),
        np.asarray(mask, np.float32),
    )
    nc = _get_nc()
    res = run_bass_kernel_spmd(nc, in_maps, core_ids=list(range(NCORES)))
    return _unshard_out([r["out"] for r in res.results])


# revision 58
# speedup vs baseline: 1.0512x; 1.0512x over previous
"""Trainium2 Bass kernel for a circular-padded 3x3 conv cellular-automaton step.

Computation (per image):
    z   = conv3x3_circular(x, Wc) ;  Wc = w1 @ w_perc  (host-fused, [96,12,3,3])
    h   = relu(z + b1)
    u   = w2 @ h + b2
    out = x + (mask > 0.5) * u

Mapping (per core, B=16 split 8 ways -> 2 images/core):
  * conv as ONE K=108 matmul per image row: partitions (dj, di, c);
    di row-shifts loaded 3x from DRAM, dj column shifts (1, 2) produced by
    two on-chip offset copies into partitions 36:72 and 72:108.
  * software-pipelined supertile loop: PE does [4 conv MMs of supertile t]
    then [4 col-tiled w2p MMs of supertile t-2], so relu latency is hidden
    and the PE streams warm (~163ns/MM) with LDWEIGHTS overlapped.
  * relu+bias split ScalarE (2 rows) / VectorE (1 row) / GPSIMD (1 row);
    ht has a constant ones-row (row 96) so b2 rides in the matmul.
  * mask multiply on VectorE against a prelaid [128, 96*384] bf16 mask,
    +x on GPSIMD, bf16 output staged per chunk, one padded DMA per chunk.
"""

import sys

if "/opt/trn_rl_repo" not in sys.path:
    sys.path.insert(0, "/opt/trn_rl_repo")

from contextlib import ExitStack

import numpy as np
import ml_dtypes

import concourse.bass as bass
import concourse.tile as tile
from concourse import mybir
from concourse.bass_utils import run_bass_kernel_spmd

B, C, H, W = 16, 12, 384, 384
CH = 96                      # hidden channels
NCORES = 8
BLOC = B // NCORES           # images per core
W2 = W + 2                   # circular-padded row length
PADH = H + 4                 # padded rows: 1 top + 3 bottom
CHUNK = 16                   # image rows per processing chunk
ST = 4                       # rows per supertile (one per PE column group)
NCHUNK = H // CHUNK
NST = CHUNK // ST            # supertiles per chunk
XQLEN = CHUNK * W2           # used free length per chunk
XBLEN = (CHUNK + 2) * W2     # loaded free length (halo + shift spill)
XPLEN = (H + 2) * W2 - 2     # flat tap-shifted x plane length per partition
MTILES = H // ST             # 96 supertile row-groups per image
STW = NST * W                # supertile-layout free length per chunk
NT = BLOC * NCHUNK * NST     # total supertiles per core

_BF16 = mybir.dt.bfloat16
_F32 = mybir.dt.float32
_FP8 = mybir.dt.float8e4


def _spill_waits(nc):
    """walrus/trn2 here accepts at most ONE sync-wait per instruction; move
    excess waits onto NoOps inserted immediately before, on the same engine."""
    nspill = 0
    for bbwrap in list(nc.bb_map.values()):
        bb = bbwrap.bb
        out = []
        for inst in bb.instructions:
            si = inst.sync_info
            if si is not None and si.on_wait and len(si.on_wait) > 1:
                waits = list(si.on_wait)
                for w in waits[1:]:
                    nop = mybir.InstNoOp(
                        name=nc.get_next_instruction_name(),
                        engine=inst.engine,
                        sync_info=mybir.SyncInfo(on_wait=[w], on_update=[]),
                        bass_nofuse=True,
                    )
                    nc.register_instruction(nop)
                    out.append(nop)
                    nspill += 1
                si.on_wait = waits[:1]
            out.append(inst)
        try:
            bb.instructions = out
        except Exception:
            bb.instructions.clear()
            bb.instructions.extend(out)
    return nspill


def _build_nc(reps=1):
    nc = bass.Bass()

    xpad = nc.declare_dram_parameter("xpad", [BLOC, 108, XPLEN], _FP8, isOutput=False)
    xstc = nc.declare_dram_parameter("xstc", [BLOC, NCHUNK // 2, ST, C, 2 * STW], _BF16, isOutput=False)
    wa = nc.declare_dram_parameter("wa", [108, CH], _BF16, isOutput=False)
    w2p = nc.declare_dram_parameter("w2p", [CH + 1, 32], _BF16, isOutput=False)
    b1 = nc.declare_dram_parameter("b1", [CH, 1], _F32, isOutput=False)
    m128 = nc.declare_dram_parameter("m128", [ST, 32, MTILES * W], _FP8, isOutput=False)
    out = nc.declare_dram_parameter("out", [BLOC, NCHUNK // 2, ST, C, 2 * STW], _BF16, isOutput=True)

    with tile.TileContext(nc) as tc, ExitStack() as ctx:
        state = _setup(ctx, tc, wa, w2p, b1, m128)
        if reps == 1:
            _loop_body(tc, state, xpad, xstc, out)
        else:
            with tc.For_i(0, reps, 1):
                _loop_body(tc, state, xpad, xstc, out)
    _spill_waits(nc)
    return nc


def _setup(ctx, tc, wa, w2p, b1, m128):
    nc = tc.nc

    const = ctx.enter_context(tc.tile_pool(name="const", bufs=1))
    ump = ctx.enter_context(tc.tile_pool(name="um", bufs=2))
    zp = ctx.enter_context(tc.tile_pool(name="z", bufs=3, space="PSUM"))
    up = ctx.enter_context(tc.tile_pool(name="u", bufs=2, space="PSUM"))

    wa_sb = const.tile([108, CH], _BF16)
    nc.sync.dma_start(out=wa_sb, in_=wa[:, :])
    w2p_sb = const.tile([CH + 1, 32], _BF16)
    nc.sync.dma_start(out=w2p_sb, in_=w2p[:, :])
    b1_sb = const.tile([CH, 1], _F32)
    nc.sync.dma_start(out=b1_sb, in_=b1[:, :])
    m128_sb = const.tile([128, MTILES * W], _FP8, name="m128_sb")

    # manually multi-buffered tiles (stable addresses); only xt needs a
    # zero memset (its pad partitions are read by the gpsimd add)
    hts = [
        const.tile([CH + 1, ST, W], _BF16, name=f"ht{i}", tag=f"ht{i}")
        for i in range(4)
    ]
    xqs = [
        const.tile([108, XBLEN], _FP8, name=f"xqt{i}", tag=f"xqt{i}") for i in range(3)
    ]
    xts = [
        const.tile([128, 2 * STW], _BF16, name=f"xtt{i}", tag=f"xtt{i}")
        for i in range(3)
    ]
    ots = [
        const.tile([128, 2 * STW], _BF16, name=f"ott{i}", tag=f"ott{i}")
        for i in range(3)
    ]
    for t in xts:
        nc.vector.memset(t, 0.0)
    for ht in hts:
        nc.vector.memset(ht[CH : CH + 1, :, :], 1.0)

    # warmup matmuls: absorb weight-load DMA waits on the PE clock
    zw = zp.tile([CH, 2, 512], _F32, tag="z2")
    nc.tensor.matmul(zw[:, 0, 0:1], wa_sb, xqs[0][:, 0:1], start=True, stop=True)
    uw = up.tile([128, 512], _F32, tag="u")
    nc.tensor.matmul(
        uw[0:32, 0:1], w2p_sb, hts[0][:, 0, 0:1], start=True, stop=True,
        tile_position=(0, 0),
    )

    return dict(
        ump=ump, zp=zp, up=up,
        wa_sb=wa_sb, w2p_sb=w2p_sb, b1_sb=b1_sb, m128_sb=m128_sb,
        m128=m128,
        hts=hts, xqs=xqs, xts=xts, ots=ots,
    )


def _loop_body(tc, state, xpad, xstc, out):
    nc = tc.nc
    add = mybir.AluOpType.add
    mult = mybir.AluOpType.mult
    amax = mybir.AluOpType.max
    relu = mybir.ActivationFunctionType.Relu
    ump, zp, up = state["ump"], state["zp"], state["up"]
    wa_sb, w2p_sb, b1_sb, m128_sb = (
        state["wa_sb"], state["w2p_sb"], state["b1_sb"], state["m128_sb"],
    )
    hts, xqs, xts, ots = state["hts"], state["xqs"], state["xts"], state["ots"]

    NCHUNKS_TOT = BLOC * NCHUNK
    LAG = 3
    ELAG = 3

    def issue_xq_dmas(ci):
        b, chk = divmod(ci, NCHUNK)
        r0 = chk * CHUNK
        xq = xqs[ci % 3]
        # ONE plain-slice DMA for all 108 (dj, di, c) partitions: the host
        # pre-lays the nine tap-shifted flat copies, so this is a regular
        # [108, L] strided load that sprays across all 16 SDMA engines
        # (custom strided APs pin every line onto a single engine).
        nc.sync.dma_start(
            out=xq[0:108, 0 : XBLEN - 2],
            in_=xpad[b, :, r0 * W2 : r0 * W2 + XBLEN - 2],
        )

    NP = NCHUNK // 2

    def issue_xt_dmas(pair):
        # xt/ot tiles stage a PAIR of chunks (32 rows) to halve DMA
        # dispatch count on the SP queue. Issued at st==3 of the first
        # chunk of the previous pair: the lag-3 adds of pair-2 (last
        # reader of this buffer) are already issued then (issue-order
        # safety - Tile WAR tracking on these tiles is not reliable)
        b, pch = divmod(pair, NP)
        xt = xts[pair % 3]
        for j in range(ST):
            nc.sync.dma_start(
                out=xt[32 * j : 32 * j + C, :], in_=xstc[b, pch, j, :, :]
            )

    # z tiles rotate through the pool; remember per-supertile handles
    zpair = {}
    upair = {}

    issue_xq_dmas(0)
    issue_xq_dmas(1)
    issue_xt_dmas(0)
    # mask load after the first-chunk x loads so they win the SP queue
    m128 = state["m128"]
    for j in range(ST):
        nc.sync.dma_start(
            out=m128_sb[32 * j : 32 * j + 32, :], in_=m128[j, :, :]
        )

    for t in range(NT + ELAG):
        ci = t // NST
        st = t % NST

        if t < NT:
            xq = xqs[ci % 3]
            z2a = zp.tile([CH, 2, 512], _F32, tag="z2")
            z2b = zp.tile([CH, 2, 512], _F32, tag="z2")
            zpair[t] = (z2a, z2b)
            for j in range(ST):
                q = st * ST + j
                zt = (z2a if j < 2 else z2b)[:, j % 2, 0:W]
                nc.tensor.matmul(
                    zt, wa_sb, xq[0:108, q * W2 : q * W2 + W],
                    start=True, stop=True,
                )

        if LAG <= t < NT + LAG:
            tp = t - LAG
            ht = hts[tp % 4]
            u = up.tile([128, 512], _F32, tag="u")
            upair[tp] = u
            for j in range(ST):
                nc.tensor.matmul(
                    u[32 * j : 32 * j + 32, 0:W], w2p_sb, ht[:, j, :],
                    start=True, stop=True, tile_position=(0, 32 * j),
                )

        if t < NT:
            # relu for supertile t: ACT rows 0,1 + 3/4 of row 2; DVE the rest
            # (balances ACT ~1466ns vs DVE ~1465ns incl. the mask op)
            z2a, z2b = zpair.pop(t)
            ht = hts[t % 4]
            nc.scalar.activation(
                out=ht[0:CH, 0:2, :], in_=z2a[:, :, 0:W], func=relu, bias=b1_sb
            )
            nc.scalar.activation(
                out=ht[0:CH, 2:3, :], in_=z2b[:, 0:1, 0:W],
                func=relu, bias=b1_sb,
            )
            nc.vector.tensor_scalar(
                ht[0:CH, 3:4, :], z2b[:, 1:2, 0:W], b1_sb, 0.0, add, amax
            )

        # elementwise (mask + add + store) for supertile t-ELAG
        if ELAG <= t < NT + ELAG:
            tq = t - ELAG
            ciq = tq // NST
            stq = tq % NST
            bq, chkq = divmod(ciq, NCHUNK)
            pq = ciq // 2
            u = upair.pop(tq)
            xt = xts[pq % 3]
            ot = ots[pq % 3]
            seg = (ciq % 2) * STW + stq * W
            tglob = chkq * NST + stq
            um = ump.tile([128, W], _BF16)
            nc.vector.tensor_tensor(
                um, u[:, 0:W], m128_sb[:, tglob * W : tglob * W + W], mult
            )
            nc.gpsimd.tensor_tensor(
                ot[:, seg : seg + W], um, xt[:, seg : seg + W], add
            )
            if ciq % 2 == 1 and stq == NST - 1:
                for j in range(ST):
                    nc.sync.dma_start(
                        out=out[bq, chkq // 2, j, :, :],
                        in_=ot[32 * j : 32 * j + C, :],
                    )

        # prefetch DMAs last: every same-buffer reader from earlier
        # chunks is already issued at this point (issue-order safety —
        # Tile's WAR tracking on these manually-buffered tiles is not
        # reliable, see the st-pattern corruption bugs)
        if t < NT:
            # xq two chunks ahead at st==1: keeps this DMA AHEAD of the
            # out-write DMAs in SP-queue order (those wait on gpsimd adds
            # and head-of-line block the queue at pair boundaries)
            if st == 1 and ci + 2 < NCHUNKS_TOT:
                issue_xq_dmas(ci + 2)
            if st == 3 and ci % 2 == 0 and ci // 2 + 1 < BLOC * NP:
                issue_xt_dmas(ci // 2 + 1)


_NC_CACHE = {}


def _get_nc():
    if "nc" not in _NC_CACHE:
        _NC_CACHE["nc"] = _build_nc()
    return _NC_CACHE["nc"]


def _prep_inputs(x, w_perc, w1, b1, w2, b2, mask):
    bf16 = ml_dtypes.bfloat16
    wc = np.einsum("hp,pcij->hcij", w1, w_perc).astype(np.float32)  # [96,12,3,3]
    # wa[36*dj + 12*di + c, h] = wc[h, c, di, dj]
    wdjdic = wc.transpose(3, 2, 1, 0)  # [dj, di, c, h]
    wa = np.ascontiguousarray(wdjdic.reshape(108, CH)).astype(bf16)
    w2p = np.zeros((CH + 1, 32), np.float32)
    w2p[0:CH, 0:C] = w2.T
    w2p[CH, 0:C] = b2
    w2p = w2p.astype(bf16)
    b1c = np.ascontiguousarray(b1.reshape(CH, 1)).astype(np.float32)

    mbit = (mask > 0.5).astype(np.float32)
    # m128c[j, c, s*W+w] = mbit[4s+j, w] for c < C, zeros on pad partitions
    m128c = np.zeros((ST, 32, MTILES * W), ml_dtypes.float8_e4m3)
    m128c[:, 0:C, :] = np.broadcast_to(
        mbit.reshape(MTILES, ST, W).transpose(1, 0, 2).reshape(ST, 1, MTILES * W),
        (ST, C, MTILES * W),
    ).astype(ml_dtypes.float8_e4m3)

    xf8 = x.astype(ml_dtypes.float8_e4m3)
    in_maps = []
    for core in range(NCORES):
        xs = x[core * BLOC : (core + 1) * BLOC]
        xsp = np.pad(
            xf8[core * BLOC : (core + 1) * BLOC],
            ((0, 0), (0, 0), (1, 3), (1, 1)),
            mode="wrap",
        )
        # nine tap-shifted flat copies: xpad9[b, 36dj+12di+c, f] =
        # flat(xsp[b, c])[dj + di*W2 + f]
        xflat = xsp.reshape(BLOC, C, PADH * W2)
        xpad9 = np.empty((BLOC, 108, XPLEN), ml_dtypes.float8_e4m3)
        for dj in range(3):
            for di in range(3):
                off = dj + di * W2
                xpad9[:, 36 * dj + 12 * di : 36 * dj + 12 * di + C, :] = (
                    xflat[:, :, off : off + XPLEN]
                )
        # pair-staged supertile x:
        # xstc[b, p, j, c, h*STW + s*W + w] = x[b, c, 32p + 16h + 4s + j, w]
        NP = NCHUNK // 2
        xstc = np.ascontiguousarray(
            xs.reshape(BLOC, C, NP, 2, NST, ST, W).transpose(0, 2, 5, 1, 3, 4, 6)
        ).reshape(BLOC, NP, ST, C, 2 * STW).astype(bf16)
        in_maps.append(
            {
                "xpad": xpad9,
                "xstc": xstc,
                "wa": wa,
                "w2p": w2p,
                "b1": b1c,
                "m128": np.ascontiguousarray(m128c),
            }
        )
    return in_maps


def _unshard_out(core_outs):
    full = np.empty((B, C, H, W), np.float32)
    for core, o in enumerate(core_outs):
        NP = NCHUNK // 2
        o = np.asarray(o, np.float32).reshape(BLOC, NP, ST, C, 2, NST, W)
        # [b, p, j, c, h, s, w] -> [b, c, (p h s j), w]
        o = o.transpose(0, 3, 1, 4, 5, 2, 6).reshape(BLOC, C, H, W)
        full[core * BLOC : (core + 1) * BLOC] = o
    return full


def kernel(x, w_perc, w1, b1, w2, b2, mask):
    x = np.asarray(x, dtype=np.float32)
    in_maps = _prep_inputs(
        x,
        np.asarray(w_perc, np.float32),
        np.asarray(w1, np.float32),
        np.asarray(b1, np.float32),
        np.asarray(w2, np.float32),
        np.asarray(b2, np.float32),
        np.asarray(mask, np.float32),
    )
    nc = _get_nc()
    res = run_bass_kernel_spmd(nc, in_maps, core_ids=list(range(NCORES)))
    return _unshard_out([r["out"] for r in res.results])
